# revision 80
# baseline (speedup 1.0000x reference)
"""Trainium2 Bass kernel for multi-head global attention (the
"DeformableAttention" module whose relative-position-bias path is inactive).

Reference computation (per batch b):
    qkv = x @ w_qkv.T + b_qkv            # [N, 3C]
    q, k, v = split/reshape to [nh, N, hd]
    attn = softmax((q @ k.T) * hd**-0.5)
    out  = (attn @ v) merged heads       # [N, C]
    y    = out @ w_proj.T + b_proj

Sharding: data-parallel over batch B=16 across 8 NeuronCores (2 batches/core).
No collectives.

Device-side design (per core, per batch), all matmul operands bf16:
  * x is staged pre-transposed (xT, [C, tokens]) so every projection
    contraction lands on SBUF partitions without on-device transpose.
  * Q^T/K^T projection uses a DENSE column layout: the 16 (q|k, head)
    96-column groups are packed into 12 full 128-wide stationary chunks
    (pair-local order), so the projection runs 12x6 matmuls per 512-token
    half instead of 16x6 at 96/128 occupancy. The per-head [96, N] Q^T/K^T
    tiles are then assembled by PSUM->SBUF copies with partition offsets
    (all ranges quadrant-aligned: starts in {0,32,64,96}).
  * V in [keys, nh*(hd+1)] layout with an interleaved ones-column per head.
  * Scores are computed transposed (S^T[k, q] blocks); exp runs on ScalarE
    with the 1/sqrt(hd) scale fused, emitting P~ in bf16.
  * PV runs in O-layout: stationary = P~ block [128k, 128q], moving =
    [V_h | 1] (97 cols, bf16) accumulating over key chunks into PSUM
    [128q, 97]; the row-sum falls out as column hd.
  * Normalization is a per-partition scale on VectorE.
  * O -> O^T via PE identity-transposes for the latency-critical batch-1
    blocks; batch 0 rides the XBAR DMA transpose path.
  * Head-loop windows are paced against the ScalarE exp stream: the next
    head-pair's projection is interleaved into the current pair's score
    loops, batch 1's first pair projects inside batch 0's last two head
    windows, and batch 0's output projection fills batch 1's last two
    windows, so the PE never drains below the exp rate.
"""

import os
import sys

sys.path.insert(0, "/opt/trn_rl_repo")

# The Bass->PJRT execution path needs jax to discover the axon-tunneled
# NeuronCores; a stray JAX_PLATFORMS=cpu (e.g. set for a jax reference run)
# would hide them. Only effective if jax hasn't been imported yet.
if "jax" not in sys.modules and "axon" not in os.environ.get("JAX_PLATFORMS", "axon"):
    os.environ.pop("JAX_PLATFORMS", None)

import numpy as np
import ml_dtypes

import concourse.bass as bass
import concourse.mybir as mybir
import concourse.tile as tile
from concourse import bacc
from concourse.bass_utils import run_bass_kernel_spmd

# Problem constants (hardcoded per the task contract).
B, N, C = 16, 1024, 768
NH, HD = 8, 96
NCORES = 8
BPC = B // NCORES  # batches per core = 2
CC = C // 128  # contraction chunks of 128 = 6
KC = N // 128  # key chunks per batch = 8
QH = N // 512  # query halves = 2
TOKC = N // 128  # token chunks for V projection = 8
QC = N // 128  # query chunks for output projection = 8
HDA = HD + 1  # head dim + ones column = 97
VW = NH * HDA  # augmented V width = 776
NPAIR = NH // 2  # head pairs = 4
MCH = 2 * NH * HD // 128  # dense QK chunks = 12
SCALE = float(HD) ** -0.5

F32 = mybir.dt.float32
BF16 = mybir.dt.bfloat16
NP_BF16 = np.dtype(ml_dtypes.bfloat16)

_BUILD_CACHE = {}


def _advance(gen, n):
    """Pull up to n quanta from a generator; returns the gen or None."""
    if gen is None:
        return None
    try:
        for _ in range(n):
            next(gen)
    except StopIteration:
        return None
    return gen


def _drain(gen):
    if gen is None:
        return
    for _ in gen:
        pass


def _build(qk_bias: bool, p_bias: bool):
    """Build + compile the single-core Bass program (shared SPMD across cores)."""
    knobs = tuple(
        int(os.environ.get(k, d))
        for k, d in (
            ("PT_BUFS", 18),
            ("QKT_BUFS", 4),
            ("SP_BUFS", 2),
            ("OP_BUFS", 2),
            ("MP_BUFS", 2),
            ("RB_BUFS", 4),
            ("OUT_BUFS", 4),
            ("O_BUFS", 14),
            ("V_BUFS", 2),
            ("WARM_MM", 16),
            ("PACE_EVEN", 3),
            ("PACE_ODD", 2),
            ("PACE_EXTRA", 2),
        )
    )
    key = (qk_bias, p_bias, knobs, os.environ.get("WARM_INJ", ""))
    if key in _BUILD_CACHE:
        return _BUILD_CACHE[key]
    (ptb, qktb, spb, opb, mpb, rbb, outb, ob, vb, warm_mm,
     pace_even, pace_odd, pace_extra) = knobs

    nc = bacc.Bacc("TRN2", target_bir_lowering=False, debug=False)

    xT_d = nc.dram_tensor("xT", [C, BPC * N], BF16, kind="ExternalInput")
    wqk_d = nc.dram_tensor("wqk", [C, 2 * NH * HD], BF16, kind="ExternalInput")
    wv_d = nc.dram_tensor("wv", [C, VW], BF16, kind="ExternalInput")
    wp_d = nc.dram_tensor("wp", [C, C], BF16, kind="ExternalInput")
    bvaug_d = nc.dram_tensor("bvaug", [1, VW], BF16, kind="ExternalInput")
    ones_d = nc.dram_tensor("ones", [1, 512], BF16, kind="ExternalInput")
    vones_d = nc.dram_tensor("vones", [128, TOKC, NH], BF16, kind="ExternalInput")
    ident_d = nc.dram_tensor("ident", [128, 128], BF16, kind="ExternalInput")
    if qk_bias:
        # dense-order flat bias row for the rank-1 bias update
        bqkf_d = nc.dram_tensor("bqkf", [1, 2 * NH * HD], BF16, kind="ExternalInput")
    if p_bias:
        bp_d = nc.dram_tensor("bp", [1, C], BF16, kind="ExternalInput")
    y_d = nc.dram_tensor("y", [BPC, N, C], F32, kind="ExternalOutput")

    xT_re = xT_d.rearrange("(o p) t -> p o t", p=128)
    wqk_re = wqk_d.rearrange("(o p) f -> p o f", p=128)
    wv_re = wv_d.rearrange("(o p) f -> p o f", p=128)
    wp_re = wp_d.rearrange("(o p) f -> p o f", p=128)

    EXP = mybir.ActivationFunctionType.Exp
    COPY = mybir.ActivationFunctionType.Copy

    with tile.TileContext(nc) as tc:
        with (
            tc.tile_pool(name="wpool", bufs=1) as wpool,
            tc.tile_pool(name="xpool", bufs=2) as xpool,
            tc.tile_pool(name="qkt_pool", bufs=qktb) as qkt_pool,
            tc.tile_pool(name="vpool", bufs=vb) as vpool,
            tc.tile_pool(name="pt_pool", bufs=ptb) as pt_pool,
            tc.tile_pool(name="attn_pool", bufs=2) as attn_pool,
            tc.tile_pool(name="rb_pool", bufs=rbb) as rb_pool,
            tc.tile_pool(name="o_pool", bufs=ob) as o_pool,
            tc.tile_pool(name="out_pool", bufs=outb) as out_pool,
            tc.tile_pool(name="spsum", bufs=spb, space="PSUM") as spsum,
            tc.tile_pool(name="opsum_pool", bufs=opb, space="PSUM") as opsum_pool,
            tc.tile_pool(name="mpsum", bufs=mpb, space="PSUM") as mpsum,
        ):
            # --- resident weights/constants ---
            # The resident wqk copy is loaded piecewise, in consumption
            # order, directly into its final tile (no separate early-chunk
            # staging tile — that would move the same bytes twice over the
            # serial DMA_ENGINES resource). Pair 0's chunks (cols 0:384)
            # lead the startup-critical stream; pair 1's (384:768) follow
            # right after the V weights; pairs 2-3 after batch 1's x.
            wqk_sb = wpool.tile([128, CC, 2 * NH * HD], BF16, tag="wqkall")
            wv_sb = wpool.tile([128, CC, VW], BF16, tag="wv")
            wv_loaded = set()

            def load_wv_lo():
                # cc-halves so batch 0's first V groups start after half
                # the bytes; emitted before x(512:1024) in the wire order.
                # Must stay on sync: a parallel-issued wv piece jumps the
                # wire FIFO ahead of the chunk/x transfers pair0 needs
                # first (measured +0.7us on the Act queue).
                if "lo" not in wv_loaded:
                    nc.sync.dma_start(wv_sb[:, 0:3, 0:512], wv_re[:, 0:3, 0:512])
                    nc.sync.dma_start(wv_sb[:, 3:CC, 0:512], wv_re[:, 3:CC, 0:512])
                    wv_loaded.add("lo")

            def load_wv_hi():
                if "hi" not in wv_loaded:
                    nc.sync.dma_start(wv_sb[:, :, 512:VW], wv_re[:, :, 512:VW])
                    wv_loaded.add("hi")
            # wp is not needed until the first output projection; issue its
            # load after the first batch's V projection to keep the
            # startup-critical DMAs (x, wv, wqk pair 0) ahead of it.
            wp_sb = wpool.tile([128, CC, C], BF16, tag="wp")
            bvaug_sb = wpool.tile([1, VW], BF16, tag="bvaug")
            ones_sb = wpool.tile([1, 512], BF16, tag="ones")
            # identity for PE transposes; first needed ~100us in
            ident_sb = wpool.tile([128, 128], BF16, tag="ident")

            def load_constants():
                # issued after the startup-critical x/wqk slices: none of
                # these is needed before ~30us in
                nc.gpsimd.dma_start(bvaug_sb[:], bvaug_d[:])
                nc.gpsimd.dma_start(ones_sb[:], ones_d[:])
                nc.gpsimd.dma_start(ident_sb[:], ident_d[:])
            if qk_bias:
                bqkf_sb = wpool.tile([1, 2 * NH * HD], BF16, tag="bqkf")
                nc.scalar.dma_start(bqkf_sb[:], bqkf_d[:])
            if p_bias:
                bp_sb = wpool.tile([1, C], BF16, tag="bp")
                nc.scalar.dma_start(bp_sb[:], bp_d[:])

            warm_box = []

            def emit_warmup():
                # PE p-state warmup: the cost model ramps the PE clock over
                # 3us of *continuous* execution (low -> mid -> full) and any
                # idle resets it. The first real matmul waits ~6.7us for its
                # DMAs; a train of dummy matmuls on a memset tile spans that
                # window so real work starts ramped, back-to-back.
                if warm_mm == 0:
                    return
                dummy = wpool.tile([128, 256], BF16, tag="warm")
                warm_box.append(dummy)
                nc.vector.memset(dummy[:], 0.0)
                wps = mpsum.tile([128, 256], F32, tag="mpsum", name="wps")
                for _ in range(warm_mm):
                    nc.tensor.matmul(
                        wps[:16, :],
                        dummy[:, 0:16],
                        dummy[:],
                        start=True,
                        stop=True,
                    )

            def stage_x0_and_wqkh0():
                """Startup-critical DMAs in need-order on the sync queue.
                chunk0 + x(0:512) are split into cc-halves, interleaved in
                consumption order, so the first matmuls start after ~half
                the bytes; the rest stream just-in-time."""
                xTb = xpool.tile([128, CC, N], BF16, tag="xTb", name="xTb")
                # chunk-0 halves ride the (empty) Act queue so they race
                # ahead of the bigger x halves on the wire — the first
                # matmuls need (wqk chunk0, x cc-half) pairs
                for cs in (slice(0, 3), slice(3, CC)):
                    nc.scalar.dma_start(
                        wqk_sb[:, cs, 0:128], wqk_re[:, cs, 0:128]
                    )
                    nc.sync.dma_start(
                        xTb[:, cs, 0 : N // 2], xT_re[:, cs, 0 : N // 2]
                    )
                nc.sync.dma_start(wqk_sb[:, :, 128:256], wqk_re[:, :, 128:256])
                nc.sync.dma_start(wqk_sb[:, :, 256:384], wqk_re[:, :, 256:384])
                return xTb

            def stage_x0b(xTb):
                # x(512:1024): emitted after the first V weights half so the
                # wire order matches the PE consumption order (pair0 tq0 ->
                # V groups t0-3 -> pair0 tq1)
                nc.sync.dma_start(xTb[:, :, N // 2 : N], xT_re[:, :, N // 2 : N])

            def pairproj_steps(g, xTb, wqk_view, box, inject=None,
                               pool_copies=False, tqs=None, tiles=None):
                """Dense Q^T/K^T projection for head pair g (heads A=2g,
                B=2g+1). Chunk layout (pair-local, set up host-side):
                  chunk 3g  : [Q_A j0:96 | K_B j0:32 ]
                  chunk 3g+1: [K_A j0:96 | K_B j32:64]
                  chunk 3g+2: [Q_B j0:96 | K_B j64:96]
                Emits 36 matmuls (one yield each) + 12 partition-shifted
                PSUM->SBUF copies. Appends (qktA, qktB) to box up front."""
                if tiles is None:
                    qktA = qkt_pool.tile([128, 2, N], BF16, tag="qkt", name="qktA")
                    qktB = qkt_pool.tile([128, 2, N], BF16, tag="qkt", name="qktB")
                    box.append((qktA, qktB))
                else:
                    qktA, qktB = tiles
                nmm = 0

                def pad_pe():
                    # startup only: dummy matmuls into an idle PSUM bank to
                    # keep the PE busy where the x/wqk DMA stream can't keep
                    # up (DMA-bandwidth-gated groups)
                    n = (inject or {}).get(nmm, 0)
                    if not n:
                        return
                    dummy = warm_box[0]  # memset tile from emit_warmup
                    wps2 = opsum_pool.tile([128, 128], F32, tag="opsum", name="pad")
                    for _ in range(n):
                        nc.tensor.matmul(
                            wps2[:16, :],
                            dummy[:, 0:16],
                            dummy[:, 0:128],
                            start=True,
                            stop=True,
                        )

                def emit_copies(mc, ts, qps):
                    # big copies on DVE; the small K_B tail pieces can ride
                    # GPSIMD (pool_copies) though DVE measures better
                    small = nc.gpsimd if pool_copies else nc.vector
                    if mc == 0:
                        nc.vector.tensor_copy(qktA[0:96, 0, ts], qps[0:96, :])
                        small.tensor_copy(qktB[0:32, 1, ts], qps[96:128, :])
                    elif mc == 1:
                        nc.vector.tensor_copy(qktA[0:96, 1, ts], qps[0:96, :])
                        small.tensor_copy(qktB[32:64, 1, ts], qps[96:128, :])
                    else:
                        nc.vector.tensor_copy(qktB[0:96, 0, ts], qps[0:96, :])
                        small.tensor_copy(qktB[64:96, 1, ts], qps[96:128, :])

                for tq in (range(QH) if tqs is None else tqs):
                    ts = slice(tq * 512, (tq + 1) * 512)
                    for mc in range(3):
                        m = 3 * g + mc
                        qps = mpsum.tile([128, 512], F32, tag="mpsum", name="qps")
                        for cc in range(CC):
                            nc.tensor.matmul(
                                qps[:, :],
                                wqk_view[:, cc, 128 * m : 128 * (m + 1)],
                                xTb[:, cc, ts],
                                start=(cc == 0),
                                stop=(cc == CC - 1 and not qk_bias),
                            )
                            nmm += 1
                            pad_pe()
                            # copies are emitted BEFORE the group's final
                            # yield: a caller that pulls exactly the yield
                            # count must still get every copy (a suspended
                            # generator never runs trailing code)
                            if cc == CC - 1 and not qk_bias:
                                emit_copies(mc, ts, qps)
                            yield
                        if qk_bias:
                            # rank-1 bias: out[c, t] += bias[c] * 1
                            nc.tensor.matmul(
                                qps[:, :],
                                bqkf_sb[:, 128 * m : 128 * (m + 1)],
                                ones_sb[:, 0:512],
                                start=False,
                                stop=True,
                            )
                            emit_copies(mc, ts, qps)
                            yield

            def emit_vproj(b, xTb, parts=None, v_sb=None, fill_ones=True):
                """Project V (ones-augmented). parts = [(lo, hi, t_iter)];
                default is half-major full projection: the (512:VW) groups
                only start after all (0:512) ones, giving the second wv
                column-half DMA slack instead of gating the second group."""
                load_wv_lo()
                if parts is None:
                    load_wv_hi()
                    parts = [(0, 512, range(TOKC)), (512, VW, range(TOKC))]
                if v_sb is None:
                    v_sb = vpool.tile([128, TOKC, VW], BF16, tag="v", name="v_sb")
                v_bias = bool(qk_bias)  # b_qkv nonzero => v bias nonzero path
                for lo, hi, t_iter in parts:
                    for t in t_iter:
                        vps = mpsum.tile([128, 512], F32, tag="mpsum", name="vps")
                        w = hi - lo
                        for cc in range(CC):
                            nc.tensor.matmul(
                                vps[:, :w],
                                xTb[:, cc, t * 128 : (t + 1) * 128],
                                wv_sb[:, cc, lo:hi],
                                start=(cc == 0),
                                stop=(cc == CC - 1 and not v_bias),
                            )
                        if v_bias:
                            # bias + per-head ones-columns via rank-1 update
                            nc.tensor.matmul(
                                vps[:, :w],
                                ones_sb[:, 0:128],
                                bvaug_sb[:, lo:hi],
                                start=False,
                                stop=True,
                            )
                        nc.vector.tensor_copy(v_sb[:, t, lo:hi], vps[:, :w])
                if fill_ones and not v_bias:
                    # fill each head's ones-column with a single strided DMA
                    nc.sync.dma_start(
                        v_sb.rearrange("p t (h a) -> p t h a", a=HDA)[:, :, :, HD],
                        vones_d[:],
                    )
                return v_sb

            def gen_proj(b, attnT, alt_queue=False):
                """Output projection as a generator: one yield per matmul."""
                for qc in range(QC):
                    out_sb = out_pool.tile([128, C], F32, tag="out", name="out_sb")
                    if alt_queue and qc == QC - 1:
                        # the very last tile: progressively narrower groups
                        # (384/256/128) so the post-last-matmul chain (copy
                        # -> DMA issue -> transfer) shrinks with the final
                        # group; the 384-group rides the scalar queue
                        subs = (
                            (0, 384, nc.scalar),
                            (384, 640, nc.sync),
                            (640, 768, nc.sync),
                        )
                    else:
                        # alternate issue queues in the pure tail so the
                        # final DMA's ~1.3us issue overhead (SEQ+HWDGE)
                        # doesn't serialize behind its predecessors
                        subs = (
                            (0, 384, nc.scalar if alt_queue else nc.sync),
                            (384, 768, nc.sync),
                        )
                    for lo, hi, q in subs:
                        w = hi - lo
                        pps = mpsum.tile([128, 512], F32, tag="mpsum", name="pps")
                        ns = slice(lo, hi)
                        for cc in range(CC):
                            nc.tensor.matmul(
                                pps[:, :w],
                                attnT[:, cc, qc * 128 : (qc + 1) * 128],
                                wp_sb[:, cc, ns],
                                start=(cc == 0),
                                stop=(cc == CC - 1 and not p_bias),
                            )
                            yield
                        if p_bias:
                            nc.tensor.matmul(
                                pps[:, :w],
                                ones_sb[:, 0:128],
                                bp_sb[:, ns],
                                start=False,
                                stop=True,
                            )
                            yield
                        # DVE only: ScalarE's queue is saturated in batch
                        # 1's h6/h7 windows (exp stream) and still draining
                        # DMA issue-waits at the tail, so Act-engine
                        # "idleness" there is not usable queue capacity
                        nc.vector.tensor_copy(out_sb[:, ns], pps[:, :w])
                        q.dma_start(
                            y_d[b, qc * 128 : (qc + 1) * 128, ns],
                            out_sb[:, ns],
                        )

            def emit_heads(b, xTb, emit_v, pair0, extra_gen=None, post_v=None):
                """Head loops for one batch. pair0 = (qktA, qktB) already
                projected. extra_gen (optional) is pulled during the last two
                head windows (h=6,7), which otherwise run below the ScalarE
                exp rate."""
                attnT = attn_pool.tile([128, CC, N], BF16, tag="attnT", name="attnT")
                o_tiles = [
                    o_pool.tile([128, C], BF16, tag="o", name=f"o{qc}")
                    for qc in range(QC)
                ]
                v_sb = emit_v()
                if post_v is not None:
                    post_v()

                def pv_step(h, qc, pts, fast_norm=False):
                    """O-layout PV for output tile (head h, query block qc).

                    fast_norm (tail only): compute the row-sum FIRST via
                    eight 1-cycle ones-matmuls into a spare PSUM column, so
                    the reciprocal runs on DVE in parallel with the big PV
                    accumulation — the post-PV chain shrinks to just the
                    tensor_scalar_mul, hiding the norm latency that
                    otherwise stalls the 2-deep opsum rotation."""
                    qh, sl = divmod(qc, KC // QH)
                    ops = opsum_pool.tile([128, 128], F32, tag="opsum", name="ops")
                    rb = rb_pool.tile([128, 1], F32, tag="rb", name="rb")
                    if fast_norm:
                        # head 0's ones-column of the resident V tile
                        onecol = v_sb[:, 0, HD : HD + 1]
                        for kc in range(KC):
                            nc.tensor.matmul(
                                ops[:, HDA : HDA + 1],
                                pts[kc][:, qh, sl * 128 : (sl + 1) * 128],
                                onecol,
                                start=(kc == 0),
                                stop=(kc == KC - 1),
                            )
                        nc.vector.reciprocal(rb[:], ops[:, HDA : HDA + 1])
                    for kc in range(KC):
                        nc.tensor.matmul(
                            ops[:, :HDA],
                            pts[kc][:, qh, sl * 128 : (sl + 1) * 128],
                            v_sb[:, kc, HDA * h : HDA * (h + 1)],
                            start=(kc == 0),
                            stop=(kc == KC - 1),
                        )
                    if not fast_norm:
                        nc.vector.reciprocal(rb[:], ops[:, HD:HDA])
                    nc.vector.tensor_scalar_mul(
                        o_tiles[qc][:, HD * h : HD * (h + 1)],
                        ops[:, :HD],
                        rb[:, 0:1],
                    )

                pair_box = [pair0]
                pair_gen = None
                prev_pts = None
                for h in range(NH):
                    g = h // 2
                    if h % 2 == 0 and g + 1 < NPAIR:
                        pair_gen = pairproj_steps(g + 1, xTb, wqk_sb, pair_box)
                    qkt_cur = pair_box[g][h % 2]
                    pts = []
                    for kc in range(KC):
                        # pacing vs the ScalarE exp cadence (~1038ns/step):
                        # h-even steps have no prev-head PV, so they need 3
                        # projection matmuls each; h-odd steps alternate 2/1
                        # so the generator spans all 8 steps (an early
                        # exhaust leaves sub-exp-rate steps that stall the
                        # PE on spsum recycling). 24 + 12 = 36 = one pair.
                        if h % 2 == 0:
                            pace = pace_even
                        else:
                            pace = pace_odd if kc % 2 == 0 else pace_odd - 1
                        st = spsum.tile([128, QH, 512], F32, tag="spsum", name="st")
                        for qh in range(QH):
                            nc.tensor.matmul(
                                st[:, qh, :],
                                qkt_cur[:HD, 1, kc * 128 : (kc + 1) * 128],
                                qkt_cur[:HD, 0, qh * 512 : (qh + 1) * 512],
                                start=True,
                                stop=True,
                            )
                        pt = pt_pool.tile([128, QH, 512], BF16, tag="pt", name="pt")
                        nc.scalar.activation(pt[:], st[:], EXP, scale=SCALE)
                        pts.append(pt)
                        pair_gen = _advance(pair_gen, pace)
                        if extra_gen is not None and h >= NH - 2:
                            # +1 on the last head's closing steps: the PE
                            # otherwise runs dry there waiting the final
                            # exps before the PV tail can start
                            boost = 1 if (h == NH - 1 and kc >= KC - 2) else 0
                            extra_gen = _advance(extra_gen, pace_extra + boost)
                        if prev_pts is not None:
                            pv_step(h - 1, kc, prev_pts)
                    if h % 2 == 1:
                        # the pair generator must be fully consumed before
                        # head 2g+2 reads its tiles — force-drain any
                        # remainder the pacing arithmetic left over
                        _drain(pair_gen)
                        pair_gen = None
                    prev_pts = pts
                # tail: PV for the last head, then O -> O^T per query block.
                # Batch 0's projection reads attnT only ~90us later, so its
                # transposes ride the XBAR DMA path off the PE — one
                # whole-tile [128, 768] -> [128, 6, 128] call per query
                # block (the 3D out AP folds the 6 chunk rows into the
                # partition dim; 8 issues instead of 48 keeps the sync
                # queue light). Batch 1's are latency-critical (the DMA
                # path stalls proj(1) on issue latency and queue credits),
                # so they use PE identity transposes with a DVE repack.
                # (fast_norm measured +1us: the 8 extra tiny matmuls per
                # block are dispatch-overhead-dominated, not 1-cycle)
                for qc in range(QC):
                    pv_step(NH - 1, qc, prev_pts)
                    if b == 0 or qc >= QC - 1:
                        nc.sync.dma_start_transpose(
                            attnT[:, :, qc * 128 : (qc + 1) * 128],
                            o_tiles[qc][:, :],
                        )
                    else:
                        tp = spsum.tile([128, CC * 128], BF16, tag="spsum", name="tp")
                        for cc in range(CC):
                            nc.tensor.transpose(
                                tp[:, cc * 128 : (cc + 1) * 128],
                                o_tiles[qc][:, cc * 128 : (cc + 1) * 128],
                                ident_sb[:],
                            )
                        nc.vector.tensor_copy(
                            attnT[:, :, qc * 128 : (qc + 1) * 128],
                            tp.rearrange("p (c t) -> p c t", t=128)[:, :, :],
                        )
                return attnT, extra_gen

            # ---- emission schedule ----
            emit_warmup()
            xTb0 = stage_x0_and_wqkh0()
            load_constants()
            xTb1_box = []

            def stage_x1_late():
                # batch 1's x + the resident wqk load ride the SAME sync
                # queue as the startup chain: DMA_ENGINES is a serial
                # resource, so a single queue in need-order guarantees the
                # startup-critical transfers (x0/wqk pair0/wv) go first.
                # (On an idle queue these would issue at t~2us and hog the
                # DMA engines ahead of the startup chain.)
                # pair 1's chunks first: they gate batch 0's head-0 window
                nc.sync.dma_start(wqk_sb[:, :, 384:768], wqk_re[:, :, 384:768])
                xTb = xpool.tile([128, CC, N], BF16, tag="xTb", name="xTb")
                for xh in range(4):
                    nc.sync.dma_start(
                        xTb[:, :, xh * (N // 4) : (xh + 1) * (N // 4)],
                        xT_re[:, :, N + xh * (N // 4) : N + (xh + 1) * (N // 4)],
                    )
                xTb1_box.append(xTb)
                nc.sync.dma_start(wqk_sb[:, :, 768:1536], wqk_re[:, :, 768:1536])

            # batch 0 pair 0: unoverlapped, straight from the early chunk
            # DMA; dummy-matmul padding absorbs the DMA-bandwidth-gated
            # stretches of the startup (counts tuned against TimelineSim)
            inj = {
                int(k): int(v)
                for k, v in (
                    kv.split(":")
                    for kv in os.environ.get("WARM_INJ", "").split(",")
                    if kv
                )
            }
            # Front interleave, matched to the DMA wire order (one serial
            # DMA_ENGINES resource): pair0-tq0 consumes [wqk c0, x 0:512,
            # c1, c2]; the first V groups (t0-3, lo half) consume wv-lo,
            # which ships before x(512:1024); pair0-tq1 then consumes
            # x(512:1024); the remaining V groups consume wv-hi.
            p0_box = []
            _drain(pairproj_steps(0, xTb0, wqk_sb, p0_box, inject=inj,
                                  pool_copies=False, tqs=(0,)))
            load_wv_lo()
            v0_sb = emit_vproj(
                0, xTb0, parts=[(0, 512, range(0, 4))], fill_ones=False
            )
            stage_x0b(xTb0)
            _drain(pairproj_steps(0, xTb0, wqk_sb, p0_box, inject=None,
                                  pool_copies=False, tqs=(1,),
                                  tiles=p0_box[0]))
            load_wv_hi()
            emit_vproj(
                0, xTb0, v_sb=v0_sb,
                parts=[(0, 512, range(4, TOKC)), (512, VW, range(TOKC))],
            )

            # batch 1 pair 0 is projected inside batch 0's h6/h7 windows;
            # it reads xTb1 via a late-bound closure (xTb1 is staged by
            # post_v long before the generator's first pull at h6).
            p0b1_box = []

            def gen_pair0_b1():
                yield from pairproj_steps(0, xTb1_box[0], wqk_sb, p0b1_box)

            at0, leftover = emit_heads(
                0, xTb0, lambda: v0_sb, p0_box[0],
                extra_gen=gen_pair0_b1(), post_v=stage_x1_late,
            )
            _drain(leftover)  # must be exhausted before b1 head 0
            xTb1 = xTb1_box[0]
            # wp is first used by proj(0), well after this point lands
            nc.sync.dma_start(wp_sb[:], wp_re[:])
            # batch 0's output projection: partially pulled into batch 1's
            # h6/h7 windows, remainder drained before proj(1)
            proj0_gen = gen_proj(0, at0)
            at1, proj0_gen = emit_heads(
                1, xTb1, lambda: emit_vproj(1, xTb1), p0b1_box[0],
                extra_gen=proj0_gen,
            )
            _drain(proj0_gen)
            _drain(gen_proj(1, at1, alt_queue=True))

    nc.compile()
    _BUILD_CACHE[key] = nc
    return nc


def _prep_shared(w_qkv, b_qkv, w_proj, b_proj):
    """Host-side weight rearrangement shared by all cores (bf16)."""
    w_qkv = np.ascontiguousarray(w_qkv, dtype=np.float32)
    w_proj = np.ascontiguousarray(w_proj, dtype=np.float32)
    b_qkv = np.asarray(b_qkv, dtype=np.float32)
    b_proj = np.asarray(b_proj, dtype=np.float32)

    # Dense pair-local QK column order; for pair g (A=2g, B=2g+1):
    #   chunk 3g  : [Q_A j0:96 | K_B j0:32 ]
    #   chunk 3g+1: [K_A j0:96 | K_B j32:64]
    #   chunk 3g+2: [Q_B j0:96 | K_B j64:96]
    # Source row of (f, h, j) in w_qkv is C*f + HD*h + j (f=0 Q, f=1 K).
    cols = []
    for g in range(NPAIR):
        A, Bh = 2 * g, 2 * g + 1
        cols += [(0, A, j) for j in range(HD)] + [(1, Bh, j) for j in range(32)]
        cols += [(1, A, j) for j in range(HD)] + [(1, Bh, j) for j in range(32, 64)]
        cols += [(0, Bh, j) for j in range(HD)] + [(1, Bh, j) for j in range(64, HD)]
    col_rows = np.array([C * f + HD * h + j for (f, h, j) in cols], dtype=np.int64)
    wqk_arr = np.ascontiguousarray(w_qkv[col_rows].T.astype(NP_BF16))  # [C, 1536]

    # wv: [C, NH*(HD+1)] with a zero ones-column slot per head
    wv = w_qkv[2 * C :].reshape(NH, HD, C)  # [h, j, c]
    wv_aug = np.zeros((C, NH, HDA), dtype=np.float32)
    wv_aug[:, :, :HD] = np.transpose(wv, (2, 0, 1))
    wv_aug = np.ascontiguousarray(wv_aug.reshape(C, VW).astype(NP_BF16))

    # wp: plain transpose [c_in, c_out]
    wp_t = np.ascontiguousarray(w_proj.T.astype(NP_BF16))

    # bvaug: v-bias interleaved with 1.0 at each head's ones-column
    bvaug = np.zeros((1, NH, HDA), dtype=np.float32)
    bvaug[0, :, :HD] = b_qkv[2 * C :].reshape(NH, HD)
    bvaug[0, :, HD] = 1.0
    bvaug = bvaug.reshape(1, VW).astype(NP_BF16)

    ones = np.ones((1, 512), dtype=NP_BF16)
    vones = np.ones((128, TOKC, NH), dtype=NP_BF16)
    ident = np.eye(128, dtype=np.float32).astype(NP_BF16)

    qk_bias = bool(np.any(b_qkv[: 2 * C] != 0.0))
    p_bias = bool(np.any(b_proj != 0.0))
    extra = {}
    if qk_bias:
        extra["bqkf"] = np.ascontiguousarray(
            b_qkv[col_rows].reshape(1, 2 * NH * HD).astype(NP_BF16)
        )
    if p_bias:
        extra["bp"] = np.ascontiguousarray(b_proj.reshape(1, C).astype(NP_BF16))

    return wqk_arr, wv_aug, wp_t, bvaug, ones, vones, ident, qk_bias, p_bias, extra


def kernel(x, w_qkv, b_qkv, w_proj, b_proj, H=32, W=32):
    x = np.asarray(x, dtype=np.float32)
    assert x.shape == (B, N, C), x.shape
    assert int(H) * int(W) == N

    wqk_arr, wv_aug, wp_t, bvaug, ones, vones, ident, qk_bias, p_bias, extra = _prep_shared(
        w_qkv, b_qkv, w_proj, b_proj
    )
    nc = _build(qk_bias, p_bias)

    in_maps = []
    for c in range(NCORES):
        xc = x[BPC * c : BPC * (c + 1)].reshape(BPC * N, C)
        xT = np.ascontiguousarray(xc.T.astype(NP_BF16))  # [C, BPC*N]
        m = {
            "xT": xT,
            "wqk": wqk_arr,
            "wv": wv_aug,
            "wp": wp_t,
            "bvaug": bvaug,
            "ones": ones,
            "vones": vones,
            "ident": ident,
        }
        m.update(extra)
        in_maps.append(m)

    trace = os.environ.get("KERNEL_TRACE") == "1"
    res = run_bass_kernel_spmd(
        nc, in_maps, core_ids=list(range(NCORES)), trace=trace
    )
    if trace:
        kernel.last_results = res
        print("exec_time_ns:", res.exec_time_ns, "mean:", res.mean_exec_time_ns)
        if res.instructions_and_trace:
            print("trace:", res.instructions_and_trace[1])

    out = np.empty((B, N, C), dtype=np.float32)
    for c in range(NCORES):
        out[BPC * c : BPC * (c + 1)] = res.results[c]["y"]
    return out


if __name__ == "__main__":
    rng = np.random.default_rng(0)
    x = rng.standard_normal((B, N, C), dtype=np.float32)
    w_qkv = rng.standard_normal((3 * C, C), dtype=np.float32) / np.sqrt(C)
    b_qkv = np.zeros(3 * C, np.float32)
    w_proj = rng.standard_normal((C, C), dtype=np.float32) / np.sqrt(C)
    b_proj = np.zeros(C, np.float32)
    y = kernel(x, w_qkv, b_qkv, w_proj, b_proj)
    print("out", y.shape, y.dtype, float(np.abs(y).mean()))
